# revision 18
# baseline (speedup 1.0000x reference)
"""ConvAttention Trainium2 kernel.

Per-core (data-parallel over batch, 8 cores, 1 image each):
  q/k/v = depthwise 3x3 conv over x [56,56,64] (+bias), then full
  attention over N=3136 tokens with softmax(q.k * 8), then ctx @ Wp + bp.

Layout strategy:
  - x is transposed on-chip (PE transposes) into a zero-padded [C=64, 58*58]
    "image" so each conv tap is a strided SBUF read.
  - convs run on the PE as diagonal-weight matmuls; k and v share one
    matmul (M=128: k rows 0-63, v rows 64-127), q separate (M=64).
  - scores are computed transposed: s^T[k_token, q_token] so softmax's
    k-reduction can be done by the AV matmul itself (ones column in v).
  - exp runs on ACT straight out of PSUM with scale=8.0 (no max pass --
    scores*8 max out around +-50, far from fp32 overflow).
  - all matmuls use float32r (~12-bit mantissa, full PE speed at N>=256).
  - normalization (1/rowsum) and +bp are folded after the Wp projection;
    final PE transpose brings the result back to [token, embed].
"""

import sys

import numpy as np

if "/opt/trn_rl_repo" not in sys.path:
    sys.path.insert(0, "/opt/trn_rl_repo")

H = 56
W = 56
C = 64
E = 64
N = H * W               # 3136 tokens
HP = H + 2              # padded
WP = W + 2
NPIX = HP * WP          # 3364
NQ = 448                # q-tile (8 spatial rows)
NQT = N // NQ           # 7
KC = 128                # k-chunk (partition dim of s^T tiles)
NKC = (N + KC - 1) // KC  # 25 (last chunk is 64 real tokens)
NPAD = NKC * KC         # 3200 (k padded with zeros)
TCH = 112               # x-transpose chunk = 2 spatial rows
NCORES = 8

_CACHE = {}


def _build(level=99):
    # level: 1=setup 2=+x-transpose 3=+kv-conv/v_nat 4=+q-conv 5=+attn 99=full
    import concourse.bacc as bacc
    import concourse.tile as tile
    from concourse import mybir
    from concourse.masks import make_identity

    F32 = mybir.dt.float32
    F32R = mybir.dt.float32r
    AF = mybir.ActivationFunctionType

    nc = bacc.Bacc(None, target_bir_lowering=False, debug=False)

    x_d = nc.dram_tensor("x", [N, C], F32, kind="ExternalInput")
    wq_d = nc.dram_tensor("wq", [9, C], F32, kind="ExternalInput")
    bq_d = nc.dram_tensor("bq", [C], F32, kind="ExternalInput")
    wk_d = nc.dram_tensor("wk", [9, C], F32, kind="ExternalInput")
    bk_d = nc.dram_tensor("bk", [C], F32, kind="ExternalInput")
    wv_d = nc.dram_tensor("wv", [9, C], F32, kind="ExternalInput")
    bv_d = nc.dram_tensor("bv", [C], F32, kind="ExternalInput")
    Wp_d = nc.dram_tensor("Wp", [C, E], F32, kind="ExternalInput")
    bp_d = nc.dram_tensor("bp", [E], F32, kind="ExternalInput")
    out_d = nc.dram_tensor("out", [N, E], F32, kind="ExternalOutput")

    with tile.TileContext(nc) as tc:
        with tc.tile_pool(name="const", bufs=1) as const, \
             tc.tile_pool(name="big", bufs=1) as big:
            # identity: build in f32 (memset/affine_select can't write f32r),
            # then round-copy to f32r for use with f32r transposes
            ident_f = const.tile([128, 128], F32)
            make_identity(nc, ident_f[:])
            ident = const.tile([128, 128], F32R)
            nc.vector.tensor_copy(ident[:], ident_f[:])
            zsc = const.tile([128, 128], F32)
            nc.vector.memset(zsc[:], 0.0)
            ones25 = const.tile([128, NKC], F32)
            nc.vector.memset(ones25[:], 1.0)

            # per-channel weights/biases as [partition, tap] scalars
            wqT = const.tile([C, 9], F32)
            nc.sync.dma_start(wqT[:], wq_d[:].transpose([1, 0]))
            wkT = const.tile([C, 9], F32)
            nc.sync.dma_start(wkT[:], wk_d[:].transpose([1, 0]))
            wvT = const.tile([C, 9], F32)
            nc.sync.dma_start(wvT[:], wv_d[:].transpose([1, 0]))
            bqT = const.tile([C, 1], F32)
            nc.sync.dma_start(bqT[:], bq_d[:].unsqueeze(1))
            bkvT = const.tile([128, 1], F32)
            nc.sync.dma_start(bkvT[0:C, :], bk_d[:].unsqueeze(1))
            nc.sync.dma_start(bkvT[C:128, :], bv_d[:].unsqueeze(1))

            # conv lhsT blocks: diagonal(w_tap)
            qw = const.tile([C, 9, C], F32R)
            kvw = const.tile([C, 9, 128], F32R)
            for t in range(9):
                nc.vector.tensor_scalar_mul(qw[:, t, :], ident[0:C, 0:C], wqT[:, t:t + 1])
                nc.vector.tensor_scalar_mul(kvw[:, t, 0:C], ident[0:C, 0:C], wkT[:, t:t + 1])
                nc.vector.tensor_scalar_mul(kvw[:, t, C:128], ident[0:C, 0:C], wvT[:, t:t + 1])

            # projection weights with bias row: [Wp; bp] (K=65)
            wp_aug = const.tile([C + 1, E], F32R)
            nc.gpsimd.dma_start(wp_aug[0:C, :], Wp_d[:])
            nc.gpsimd.dma_start(wp_aug[C:C + 1, :], bp_d[:].unsqueeze(0))

            # stage x: [112, 28, 64] via 4 chunked HWDGE loads so the first
            # transpose starts early (cast to f32r happens in the PSUM->SBUF copy)
            xstage = big.tile([TCH, N // TCH, C], F32)
            xsrc = x_d[:].rearrange("(r p) c -> p r c", p=TCH)
            for dc in range(4):
                nc.sync.dma_start(xstage[:, dc * 7:(dc + 1) * 7, :],
                                  xsrc[:, dc * 7:(dc + 1) * 7, :])

            # big persistent tensors
            xpT = big.tile([C, HP, WP], F32R)          # padded transposed image
            qT = big.tile([C, N], F32R)                # q^T  [c, token]
            kvT = big.tile([128, NPAD], F32R)          # rows 0-63 k^T, 64-127 v^T
            v_nat = big.tile([128, NKC, C + 1], F32R)  # [token%128, chunk, c|ones]
            # final-stage transpose staging: rows 0-63 proj^T, row 64 rowsum,
            # rows 65-95 zero padding (PE transpose needs K % 32 == 0)
            t2a = big.tile([96, NQ], F32)
            t2b = big.tile([96, NQ], F32)
            nc.vector.memset(t2a[64:96, :], 0.0)
            nc.vector.memset(t2b[64:96, :], 0.0)

            # zero-fill f32r regions via f32->f32r copies (memset can't emit f32r):
            # xpT border rows/cols, kvT's k-token padding, and the garbage
            # upper half of v_nat's last (64-token) chunk; ones column for
            # the rowsum trick.
            nc.vector.tensor_copy(xpT[:, 0, :], zsc[0:C, 0:WP])
            nc.vector.tensor_copy(xpT[:, HP - 1, :], zsc[0:C, 0:WP])
            nc.vector.tensor_copy(xpT[:, :, 0:1], zsc[0:C, 0:HP].unsqueeze(2))
            nc.vector.tensor_copy(xpT[:, :, WP - 1:WP], zsc[0:C, 0:HP].unsqueeze(2))
            nc.vector.tensor_copy(kvT[:, N:NPAD], zsc[:, 0:NPAD - N])
            nc.vector.tensor_copy(v_nat[:, :, C], ones25[:])
            nc.vector.tensor_copy(v_nat[C:128, NKC - 1, :], zsc[C:128, 0:C + 1])

            with tc.tile_pool(name="ps1", bufs=2, space="PSUM") as ps1, \
                 tc.tile_pool(name="ps1b", bufs=4, space="PSUM") as ps1b, \
                 tc.tile_pool(name="ps1c", bufs=2, space="PSUM") as ps1c:
                # x -> xpT (PE transpose 2 spatial rows at a time); copies
                # alternate between DVE and ACT to halve the copy wall-time
                for r in range(N // TCH if level >= 2 else 0):
                    pt = ps1b.tile([C, TCH], F32, tag="tp")
                    nc.tensor.transpose(pt[:], xstage[:, r, :], ident_f[0:TCH, 0:TCH])
                    dst = xpT[:, 1 + 2 * r:3 + 2 * r, 1:1 + W]
                    src = pt[:].rearrange("c (h w) -> c h w", w=W)
                    if r % 2 == 0:
                        nc.vector.tensor_copy(dst, src)
                    else:
                        nc.scalar.copy(dst, src)

                # k+v convs (paired, M=128)
                for ct in range(NQT if level >= 3 else 0):
                    pkv = ps1.tile([128, NQ], F32, tag="cv")
                    for t in range(9):
                        i, j = t // 3, t % 3
                        nc.tensor.matmul(
                            pkv[:], kvw[:, t, :],
                            xpT[:, ct * 8 + i:ct * 8 + i + 8, j:j + W],
                            start=(t == 0), stop=(t == 8))
                    nc.vector.tensor_scalar_add(
                        kvT[:, ct * NQ:(ct + 1) * NQ], pkv[:], bkvT[:, 0:1])

                # v^T -> v_nat (PE transpose, 128-token chunks)
                for kc in range(NKC if level >= 3 else 0):
                    cw = min(KC, N - kc * KC)
                    tp = ps1c.tile([128, C], F32R, tag="tp2")
                    nc.tensor.transpose(
                        tp[0:cw, :], kvT[C:128, kc * KC:kc * KC + cw],
                        ident[C:128, C:128])
                    if kc % 2 == 0:
                        nc.vector.tensor_copy(v_nat[0:cw, kc, 0:C], tp[0:cw, :])
                    else:
                        nc.scalar.copy(v_nat[0:cw, kc, 0:C], tp[0:cw, :])

            with tc.tile_pool(name="ps2", bufs=1, space="PSUM") as ps2, \
                 tc.tile_pool(name="psS", bufs=2, space="PSUM") as psS, \
                 tc.tile_pool(name="psC", bufs=1, space="PSUM") as psC, \
                 tc.tile_pool(name="psF", bufs=2, space="PSUM") as psF, \
                 tc.tile_pool(name="sbA", bufs=3) as sbA, \
                 tc.tile_pool(name="sbB", bufs=2) as sbB:
                for qt in range(NQT if level >= 4 else 0):
                    q0 = qt * NQ
                    # q conv for this tile
                    pq = ps2.tile([C, NQ], F32, tag="qcv")
                    for t in range(9):
                        i, j = t // 3, t % 3
                        nc.tensor.matmul(
                            pq[:], qw[:, t, :],
                            xpT[:, qt * 8 + i:qt * 8 + i + 8, j:j + W],
                            start=(t == 0), stop=(t == 8))
                    nc.vector.tensor_scalar_add(
                        qT[:, q0:q0 + NQ], pq[:], bqT[:, 0:1])

                    if level < 5:
                        continue
                    # attention: s^T chunks -> exp -> AV accumulate
                    pctx = psC.tile([C + 1, NQ], F32, tag="ctx")
                    for b in range((NKC + 1) // 2):
                        nb = min(2, NKC - b * 2)
                        # last chunk holds only 64 real k-tokens; shrink it
                        pw = 64 if (b * 2 + nb) == NKC else 128
                        ps_s = psS.tile([128, 2, 512], F32, tag="s")
                        pT = sbA.tile([128, 2, NQ], F32R, tag="p")
                        for jj in range(nb):
                            kc = b * 2 + jj
                            cw = 64 if kc == NKC - 1 else 128
                            nc.tensor.matmul(
                                ps_s[0:cw, jj, 0:NQ],
                                kvT[0:C, kc * KC:kc * KC + cw],
                                qT[:, q0:q0 + NQ],
                                start=True, stop=True)
                        nc.scalar.activation(
                            pT[0:pw, 0:nb, :], ps_s[0:pw, 0:nb, 0:NQ],
                            AF.Exp, scale=8.0)
                        for jj in range(nb):
                            kc = b * 2 + jj
                            cw = 64 if kc == NKC - 1 else 128
                            nc.tensor.matmul(
                                pctx[:], v_nat[0:cw, kc, :], pT[0:cw, jj, :],
                                start=(kc == 0), stop=(kc == NKC - 1))

                    if level < 6:
                        continue
                    ctxT = sbB.tile([C + 1, NQ], F32R, tag="ctxT")
                    nc.vector.tensor_copy(ctxT[:], pctx[:])

                    pp2 = psF.tile([128, NQ], F32, tag="fin")
                    nc.tensor.matmul(pp2[0:E, :], wp_aug[:], ctxT[:],
                                     start=True, stop=True)
                    t2 = t2a if qt % 2 == 0 else t2b
                    nc.vector.tensor_copy(t2[0:E, :], pp2[0:E, :])
                    nc.vector.tensor_copy(t2[C:C + 1, :], ctxT[C:C + 1, :])

                    fin = sbB.tile([TCH, 4, E], mybir.dt.float32, tag="fin4")
                    for c4 in range(4):
                        pf = psF.tile([128, 96], F32, tag="fin")
                        nc.tensor.transpose(
                            pf[0:TCH, :], t2[:, c4 * TCH:(c4 + 1) * TCH],
                            ident_f[0:96, 0:96])
                        inv = sbB.tile([TCH, 1], mybir.dt.float32, tag="inv")
                        nc.vector.reciprocal(inv[:], pf[0:TCH, C:C + 1])
                        nc.vector.tensor_scalar_mul(
                            fin[:, c4, :], pf[0:TCH, 0:E], inv[:, 0:1])
                    nc.sync.dma_start(
                        out_d[q0:q0 + NQ, :].rearrange("(c p) e -> p c e", p=TCH),
                        fin[:])

    nc.compile()
    return nc


def _get_nc():
    if "nc" not in _CACHE:
        _CACHE["nc"] = _build()
    return _CACHE["nc"]


def kernel(x, wq, bq, wk, bk, wv, bv, Wp, bp):
    from concourse.bass_utils import run_bass_kernel_spmd

    nc = _get_nc()
    x = np.ascontiguousarray(np.asarray(x, dtype=np.float32))
    shared = {
        "wq": np.ascontiguousarray(np.asarray(wq, np.float32).reshape(9, C)),
        "bq": np.ascontiguousarray(np.asarray(bq, np.float32)),
        "wk": np.ascontiguousarray(np.asarray(wk, np.float32).reshape(9, C)),
        "bk": np.ascontiguousarray(np.asarray(bk, np.float32)),
        "wv": np.ascontiguousarray(np.asarray(wv, np.float32).reshape(9, C)),
        "bv": np.ascontiguousarray(np.asarray(bv, np.float32)),
        "Wp": np.ascontiguousarray(np.asarray(Wp, np.float32)),
        "bp": np.ascontiguousarray(np.asarray(bp, np.float32)),
    }
    in_maps = [dict(shared, x=x[i].reshape(N, C)) for i in range(NCORES)]
    res = run_bass_kernel_spmd(nc, in_maps, core_ids=list(range(NCORES)))
    out = np.stack([res.results[i]["out"].reshape(H, W, E) for i in range(NCORES)])
    return out
